# revision 21
# baseline (speedup 1.0000x reference)
"""LGCN (3-layer edge-weighted graph conv, concat features) on 8 TRN2 NeuronCores.

Strategy (graph-partition sharding per spec hint):
- Nodes sharded across 8 cores (12544 = 98x128 rows each); each core owns the
  edges whose dst falls in its shard.
- Per layer: per-edge feature rows are gathered from a replicated HBM node
  table via dma_gather (int16 indices; src space split into 4 chunks of 25088
  rows to fit int16), messages scaled by edge weight on DVE, and scattered
  into the owned node block via a one-hot matmul accumulated in PSUM
  (dst-major edge ordering makes each 128-node block a PSUM accumulation
  group).
- Between layers the computed node shard is AllGather'd into every core's
  node table (halo exchange degenerates to full replication for this
  locality-free random graph).

Host-side preprocessing (numpy) builds the per-core edge arrays (gather
indices, one-hot keys, weights) and a core-shared static loop structure
(tile counts are maxed across cores so the single SPMD program fits all 8
data sets).
"""

import math
import sys

sys.path.insert(0, "/opt/trn_rl_repo")

import numpy as np
import ml_dtypes

from concourse import bass, bacc, mybir, tile
from concourse.bass import AP
from concourse.bass_utils import run_bass_kernel_spmd

P = 128          # SBUF partitions
BLK = 128        # nodes per dst block (PSUM partition dim)
DP = 128         # padded feature columns (bf16) -> 256B gather rows
CH = 4           # src chunks (int16 gather index range)
SLOTS_PER_BANK = 7   # 7 x 64 f32 = 1792B < 2KB PSUM bank
MAX_GRP_BLOCKS = 56  # blocks per drain group (8 banks x 7)
TB = 8           # tiles (128 edges) per gather/compute batch (>8 hangs HW DGE)

BF16 = mybir.dt.bfloat16
F32 = mybir.dt.float32
I16 = mybir.dt.int16

SKIP_COLLECTIVES = False  # hang-bisection switch (test only)
NQ = 4               # SWDGE queues; cycling gathers across queues overlaps
                     # desc-gen with per-queue DMA waits (4.3x gather rate)


class Cfg:
    def __init__(self, n_nodes, d_feat, n_layers, n_cores):
        self.N = n_nodes
        self.D = d_feat
        self.L = n_layers
        self.NC = n_cores
        self.SHARD = int(math.ceil(math.ceil(n_nodes / n_cores) / BLK)) * BLK
        self.BPC = self.SHARD // BLK                   # blocks per core
        self.NG = int(math.ceil(self.BPC / MAX_GRP_BLOCKS))   # drain groups
        self.GBLK = int(math.ceil(self.BPC / self.NG))        # blocks per group
        self.TBL_ROWS = self.NC * self.SHARD
        assert self.TBL_ROWS % CH == 0
        self.CHUNK_R = self.TBL_ROWS // CH
        assert self.CHUNK_R <= 32768, "int16 gather index overflow"
        self.DO = (n_layers + 1) * d_feat              # output cols
        # split-allgather: drain-group slices of every core land contiguously
        # in the table so each per-group collective unlocks a chunk pair.
        self.HALF = self.GBLK * BLK
        self.SPLIT = (
            self.NG * self.GBLK == self.BPC
            and (self.NC * self.HALF) % self.CHUNK_R == 0
        )

    def table_row(self, node):
        """Global node id -> (possibly permuted) replicated-table row."""
        if not self.SPLIT:
            return node
        r = node // self.SHARD
        j = node % self.SHARD
        g = j // self.HALF
        return g * (self.NC * self.HALF) + r * self.HALF + (j % self.HALF)


class Plan:
    """Core-shared static structure: segment tile counts and emission order."""

    def __init__(self, cfg, seg_tiles):
        # seg_tiles[g][c][b] : tiles for (group, chunk, block-in-group)
        self.cfg = cfg
        self.seg_tiles = seg_tiles
        self.T_total = int(seg_tiles.sum())
        # tile -> (g, c, b) in emission order (g-major, then c, then b)
        self.tiles = []
        self.spans = {}   # (g, c) -> (t0, t1)
        # gather calls: one per <=TB-tile slice of a (g,c,b) segment so that
        # per-core pad lanes (idx=-1) always trail their call and generate no
        # DMA descriptors (num_idxs_reg carries the per-core valid count).
        self.calls = []   # (t0, nt, g, c, b, seg_lane0); seg_lane0 = call's
                          # first lane offset within its segment
        t = 0
        for g in range(cfg.NG):
            for c in range(CH):
                t0 = t
                for b in range(self._gblocks(g)):
                    st = int(seg_tiles[g, c, b])
                    off = 0
                    while off < st:
                        nt = min(TB, st - off)
                        self.calls.append((t, nt, g, c, b, off * BLK))
                        for _ in range(nt):
                            self.tiles.append((g, c, b))
                            t += 1
                        off += nt
                self.spans[(g, c)] = (t0, t)
        self.call_index = {(c[0], c[1]): i for i, c in enumerate(self.calls)}
        # first/last tile per (g, bank) for start/stop flags
        self.first_of_bank = {}
        self.last_of_bank = {}
        for t, (g, c, b) in enumerate(self.tiles):
            key = (g, b // SLOTS_PER_BANK)
            if key not in self.first_of_bank:
                self.first_of_bank[key] = t
            self.last_of_bank[key] = t

    def _gblocks(self, g):
        cfg = self.cfg
        return min(cfg.GBLK, cfg.BPC - g * cfg.GBLK)

    def gblocks(self, g):
        return self._gblocks(g)

    def banks(self, g):
        return int(math.ceil(self._gblocks(g) / SLOTS_PER_BANK))


def _exclusive_cumsum(a):
    out = np.zeros_like(a)
    out[1:] = np.cumsum(a)[:-1]
    return out


def preprocess(x, src, dst, w, cfg):
    """Build per-core input maps and the shared Plan."""
    N, NC, SHARD, BPC, NG, GBLK = cfg.N, cfg.NC, cfg.SHARD, cfg.BPC, cfg.NG, cfg.GBLK
    D = cfg.D

    core = dst // SHARD
    blk = (dst % SHARD) // BLK
    grp = blk // GBLK
    b_in_g = blk - grp * GBLK
    trow = cfg.table_row(src)
    chunk = trow // cfg.CHUNK_R
    dst_rel = dst % BLK

    nkeys = NG * CH * GBLK
    key = (grp * CH + chunk) * GBLK + b_in_g       # per-core segment key
    counts = np.zeros((NC, nkeys), dtype=np.int64)
    for r in range(NC):
        counts[r] = np.bincount(key[core == r], minlength=nkeys)

    seg_tiles = -(-counts.max(axis=0) // BLK).reshape(NG, CH, GBLK)
    # blocks beyond BPC in the last group must have 0 tiles
    for g in range(NG):
        nb = min(GBLK, BPC - g * GBLK)
        seg_tiles[g, :, nb:] = 0
    # every real block needs >=1 tile so its PSUM slot is written
    for g in range(NG):
        nb = min(GBLK, BPC - g * GBLK)
        empty = seg_tiles[g].sum(axis=0)[:nb] == 0
        seg_tiles[g, 0, :nb][empty] = 1

    plan = Plan(cfg, seg_tiles)
    seg_edges = (seg_tiles * BLK).reshape(-1)
    seg_start = _exclusive_cumsum(seg_edges)
    E_pad = int(seg_edges.sum())
    T = plan.T_total
    assert E_pad == T * BLK

    iota = np.tile(np.arange(P, dtype=np.float32)[None, :], (P, 1)).astype(
        ml_dtypes.bfloat16
    )

    x_tbl = np.zeros((cfg.TBL_ROWS, DP), dtype=ml_dtypes.bfloat16)
    x_tbl[cfg.table_row(np.arange(N)), :D] = x.astype(ml_dtypes.bfloat16)

    in_maps = []
    for r in range(NC):
        sel = core == r
        s_key = key[sel]
        s_trow = trow[sel]
        s_chunk = chunk[sel]
        s_dst_rel = dst_rel[sel]
        s_w = w[sel]

        order = np.argsort(s_key, kind="stable")
        sk = s_key[order]
        kcnt = np.bincount(sk, minlength=nkeys)
        kstart = _exclusive_cumsum(kcnt)
        rank = np.arange(len(sk)) - kstart[sk]
        pos = seg_start[sk] + rank

        # pad lanes carry idx=-1: they trail each gather call, so the DGE
        # skips them (no descriptor, no packet); num_idxs_reg = valid count
        idx16 = np.full(E_pad, -1, dtype=np.int16)
        idx16[pos] = (s_trow[order] - s_chunk[order] * cfg.CHUNK_R).astype(np.int16)
        dstrel = np.full(E_pad, -1.0, dtype=np.float32)
        dstrel[pos] = s_dst_rel[order].astype(np.float32)
        warr = np.zeros(E_pad, dtype=np.float32)
        warr[pos] = s_w[order]

        # per-call valid-index counts (>=1: a zero-count call keeps one
        # keepalive lane with idx=0, w=0, dstrel=-1)
        cnt32 = np.zeros(len(plan.calls), dtype=np.int32)
        for ci, (t0, nt, g, c, b, lane0) in enumerate(plan.calls):
            k = (g * CH + c) * GBLK + b
            v = int(np.clip(kcnt[k] - lane0, 0, nt * BLK))
            if v == 0:
                idx16[seg_start[k] + lane0] = 0
                v = 1
            cnt32[ci] = v

        idx_pack = np.tile(idx16.reshape(-1, 16).T, (8, 1))      # [128, T*8]
        dst_pack = dstrel.reshape(T, BLK).T.astype(ml_dtypes.bfloat16)  # [128, T]
        w_pack = warr.reshape(T, BLK).T.astype(ml_dtypes.bfloat16)      # [128, T]

        xs = np.zeros((SHARD, D), dtype=np.float32)
        lo = r * SHARD
        hi = min(N, lo + SHARD)
        if hi > lo:
            xs[: hi - lo] = x[lo:hi]

        in_maps.append(
            {
                "x_tbl": x_tbl,
                "xs": xs,
                "idx": np.ascontiguousarray(idx_pack),
                "dstv": np.ascontiguousarray(dst_pack),
                "wv": np.ascontiguousarray(w_pack),
                "iota": iota,
                "cnt": cnt32.reshape(1, -1),
            }
        )
    return in_maps, plan


def build(cfg, plan):
    """Build the SPMD Bass program (same instruction stream for all cores)."""
    NC, D, T = cfg.NC, cfg.D, plan.T_total
    nc = bacc.Bacc("TRN2", target_bir_lowering=False, debug=False, num_devices=NC,
                   num_swdge_queues=NQ, dynamic_dma_scratch_size=49152)

    x_tbl = nc.dram_tensor("x_tbl", [cfg.TBL_ROWS, DP], BF16, kind="ExternalInput")
    xs = nc.dram_tensor("xs", [cfg.SHARD, D], F32, kind="ExternalInput")
    idx_d = nc.dram_tensor("idx", [P, T * 8], I16, kind="ExternalInput")
    dst_d = nc.dram_tensor("dstv", [P, T], BF16, kind="ExternalInput")
    w_d = nc.dram_tensor("wv", [P, T], BF16, kind="ExternalInput")
    iota_d = nc.dram_tensor("iota", [P, P], BF16, kind="ExternalInput")
    cnt_d = nc.dram_tensor("cnt", [1, len(plan.calls)], mybir.dt.int32,
                           kind="ExternalInput")
    out_d = nc.dram_tensor("out", [cfg.SHARD, cfg.DO], F32, kind="ExternalOutput")

    shards = [
        nc.dram_tensor(f"hshard{l}", [cfg.SHARD, DP], BF16)
        for l in range(cfg.L - 1)
    ]
    tbls = [
        nc.dram_tensor(f"htbl{l}", [cfg.TBL_ROWS, DP], BF16, addr_space="Shared")
        for l in range(cfg.L - 1)
    ]

    core_ids = list(range(NC))

    with tile.TileContext(nc, num_cores=NC) as tc:
        with tc.tile_pool(name="consts", bufs=1) as consts, \
             tc.tile_pool(name="work", bufs=6) as work, \
             tc.tile_pool(name="stage", bufs=2) as stage, \
             tc.tile_pool(name="ps", bufs=8, space="PSUM") as ps:

            idx_sb = consts.tile([P, T * 8], I16)
            dst_sb = consts.tile([P, T], BF16)
            w_sb = consts.tile([P, T], BF16)
            iota_sb = consts.tile([P, P], BF16)
            cnt_sb = consts.tile([1, len(plan.calls)], mybir.dt.int32)
            nc.sync.dma_start(idx_sb[:], idx_d[:])
            nc.sync.dma_start(dst_sb[:], dst_d[:])
            nc.sync.dma_start(w_sb[:], w_d[:])
            nc.sync.dma_start(iota_sb[:], iota_d[:])
            nc.sync.dma_start(cnt_sb[:], cnt_d[:])

            # gather outputs keep stale lanes beyond the valid count; memset
            # the rotating buffers once so stale bits are never NaN (w=0 and
            # zero one-hot columns only kill finite garbage)
            for _ in range(6):
                mg0 = work.tile([P, TB, DP], BF16, tag="mg")
                nc.vector.memset(mg0[:], 0.0)

            # one shared count register: gpsimd executes in order, so
            # load->gather pairs never overlap a stale value
            cnt_reg = nc.gpsimd.alloc_register("cnt_reg")

            # one-time zero of shard pad columns (collective reads full rows)
            zpad = consts.tile([P, cfg.BPC, D], BF16)
            nc.vector.memset(zpad[:], 0.0)
            for sh in shards:
                nc.sync.dma_start(
                    AP(sh, D, [[DP, P], [BLK * DP, cfg.BPC], [1, D]]),
                    zpad[:],
                )

            # out[:, 0:D] = x shard (bounce through SBUF)
            xb = consts.tile([P, cfg.BPC, D], F32)
            nc.sync.dma_start(
                xb[:],
                AP(xs, 0, [[D, P], [BLK * D, cfg.BPC], [1, D]]),
            )
            nc.sync.dma_start(
                AP(out_d, 0, [[cfg.DO, P], [BLK * cfg.DO, cfg.BPC], [1, D]]),
                xb[:],
            )

            gq = 0  # gather batch counter for queue cycling
            for l in range(cfg.L):
                src_tbl = x_tbl if l == 0 else tbls[l - 1]
                for g in range(cfg.NG):
                    psum_tiles = []
                    for pt in range(plan.banks(g)):
                        psum_tiles.append(
                            ps.tile([P, SLOTS_PER_BANK * D], F32, space="PSUM",
                                    tag="ps", name=f"ps_{l}_{g}_{pt}")
                        )
                    # greedy-pack consecutive per-segment gather calls into
                    # <=TB-tile super-batches: gathers stay per-segment (pad
                    # lanes trail + reg count skips them), DVE/matmuls run on
                    # the full batch
                    gcalls = [cl for cl in plan.calls if cl[2] == g]
                    bi = 0
                    while bi < len(gcalls):
                        batch = [gcalls[bi]]
                        bt = gcalls[bi][1]
                        bi += 1
                        while bi < len(gcalls) and bt + gcalls[bi][1] <= TB:
                            batch.append(gcalls[bi])
                            bt += gcalls[bi][1]
                            bi += 1
                        bt0 = batch[0][0]

                        mg = work.tile([P, TB, DP], BF16, tag="mg")
                        s_eq = work.tile([P, TB, P], BF16, tag="seq")
                        mw = work.tile([P, TB, D], BF16, tag="mw")

                        for (tt, nt, cg, c, cb, lane0) in batch:
                            ci = plan.call_index[(tt, nt)]
                            off = tt - bt0
                            nc.gpsimd.reg_load(cnt_reg, cnt_sb[0:1, ci:ci + 1])
                            nc.gpsimd.dma_gather(
                                out_ap=mg[:, off:off + nt, :],
                                in_ap=src_tbl[c * cfg.CHUNK_R:(c + 1) * cfg.CHUNK_R, :],
                                idxs_ap=idx_sb[:, tt * 8:(tt + nt) * 8],
                                num_idxs=nt * BLK,
                                num_idxs_reg=cnt_reg,
                                elem_size=DP,
                                queue_num=gq % NQ,
                            )
                            gq += 1

                        iota_ap = iota_sb[:]
                        iota_b = AP(
                            iota_ap.tensor, iota_ap.offset,
                            [list(iota_ap.ap[0]), [0, bt], [1, P]],
                        )
                        dslice = dst_sb[:, bt0:bt0 + bt]
                        dst_b = AP(
                            dslice.tensor, dslice.offset,
                            [list(dslice.ap[0]), [1, bt], [0, P]],
                        )
                        nc.vector.tensor_tensor(
                            out=s_eq[:, 0:bt, :], in0=iota_b, in1=dst_b,
                            op=mybir.AluOpType.is_equal,
                        )

                        wslice = w_sb[:, bt0:bt0 + bt]
                        w_b = AP(
                            wslice.tensor, wslice.offset,
                            [list(wslice.ap[0]), [1, bt], [0, D]],
                        )
                        nc.vector.tensor_tensor(
                            out=mw[:, 0:bt, :], in0=mg[:, 0:bt, 0:D], in1=w_b,
                            op=mybir.AluOpType.mult,
                        )

                        for k in range(bt):
                            t = bt0 + k
                            _, _, b = plan.tiles[t]
                            pt, slot = b // SLOTS_PER_BANK, b % SLOTS_PER_BANK
                            nc.tensor.matmul(
                                out=psum_tiles[pt][:, slot * D:(slot + 1) * D],
                                lhsT=s_eq[:, k, :],
                                rhs=mw[:, k, :],
                                start=(plan.first_of_bank[(g, pt)] == t),
                                stop=(plan.last_of_bank[(g, pt)] == t),
                                skip_group_check=True,
                            )

                    # drains
                    for pt in range(plan.banks(g)):
                        nb = min(SLOTS_PER_BANK, plan.gblocks(g) - pt * SLOTS_PER_BANK)
                        row0 = (g * cfg.GBLK + pt * SLOTS_PER_BANK) * BLK
                        o_st = stage.tile([P, SLOTS_PER_BANK * D], F32, tag="ost")
                        nc.scalar.copy(o_st[:, 0:nb * D], psum_tiles[pt][:, 0:nb * D])
                        nc.sync.dma_start(
                            AP(out_d, row0 * cfg.DO + (l + 1) * D,
                               [[cfg.DO, P], [BLK * cfg.DO, nb], [1, D]]),
                            AP(o_st.tensor, o_st[:].offset,
                               [list(o_st[:].ap[0]), [D, nb], [1, D]]),
                        )
                        if l < cfg.L - 1:
                            h_st = stage.tile([P, SLOTS_PER_BANK * D], BF16, tag="hst")
                            nc.scalar.copy(
                                h_st[:, 0:nb * D], psum_tiles[pt][:, 0:nb * D]
                            )
                            nc.sync.dma_start(
                                AP(shards[l], row0 * DP,
                                   [[DP, P], [BLK * DP, nb], [1, D]]),
                                AP(h_st.tensor, h_st[:].offset,
                                   [list(h_st[:].ap[0]), [D, nb], [1, D]]),
                            )

                    # per-group-piece allgather: overlaps the next group's
                    # compute and unlocks the next layer's chunk pair early
                    if l < cfg.L - 1 and cfg.SPLIT and not SKIP_COLLECTIVES:
                        nc.gpsimd.collective_compute(
                            "AllGather",
                            mybir.AluOpType.bypass,
                            replica_groups=[core_ids],
                            ins=[shards[l][g * cfg.HALF:(g + 1) * cfg.HALF, :]],
                            outs=[tbls[l][g * cfg.NC * cfg.HALF:
                                          (g + 1) * cfg.NC * cfg.HALF, :]],
                        )

                if l < cfg.L - 1 and not cfg.SPLIT and not SKIP_COLLECTIVES:
                    nc.gpsimd.collective_compute(
                        "AllGather",
                        mybir.AluOpType.bypass,
                        replica_groups=[core_ids],
                        ins=[shards[l][:]],
                        outs=[tbls[l][:]],
                    )

    nc.compile()
    return nc


class _FastResults:
    def __init__(self, results):
        self.results = results
        self.exec_time_ns = None


class _FastRunner:
    """Caches the compiled PJRT executable and device-resident inputs so
    repeat executions only dispatch + execute (no host concat/upload)."""

    def __init__(self, nc, in_maps, n_cores):
        import jax
        import jax.numpy as jnp
        from jax.sharding import Mesh, PartitionSpec, NamedSharding
        from jax.experimental.shard_map import shard_map
        from concourse import bass2jax

        bass2jax.install_neuronx_cc_hook()
        self._jax = jax
        partition_name = (
            nc.partition_id_tensor.name if nc.partition_id_tensor else None
        )
        in_names, out_names, out_avals, zero_shapes = [], [], [], []
        for alloc in nc.m.functions[0].allocations:
            if not isinstance(alloc, mybir.MemoryLocationSet):
                continue
            name = alloc.memorylocations[0].name
            if alloc.kind == "ExternalInput":
                if name != partition_name:
                    in_names.append(name)
            elif alloc.kind == "ExternalOutput":
                shape = tuple(alloc.tensor_shape)
                dtype = mybir.dt.np(alloc.dtype)
                out_names.append(name)
                out_avals.append(jax.core.ShapedArray(shape, dtype))
                zero_shapes.append((shape, dtype))
        n_params = len(in_names)
        n_outs = len(out_names)
        names_all = list(in_names) + list(out_names)
        if partition_name is not None:
            names_all.append(partition_name)
        donate = tuple(range(n_params, n_params + n_outs))
        self._out_names = out_names
        self._out_avals = out_avals
        self._n_cores = n_cores

        def _body(*args):
            operands = list(args)
            if partition_name is not None:
                operands.append(bass2jax.partition_id_tensor())
            outs = bass2jax._bass_exec_p.bind(
                *operands,
                out_avals=tuple(out_avals),
                in_names=tuple(names_all),
                out_names=tuple(out_names),
                lowering_input_output_aliases=(),
                sim_require_finite=True,
                sim_require_nnan=True,
                nc=nc,
            )
            return tuple(outs)

        devices = jax.devices()[:n_cores]
        mesh = Mesh(np.asarray(devices), ("core",))
        sh = NamedSharding(mesh, PartitionSpec("core"))
        in_specs = (PartitionSpec("core"),) * (n_params + n_outs)
        out_specs = (PartitionSpec("core"),) * n_outs
        self._sharded = jax.jit(
            shard_map(_body, mesh=mesh, in_specs=in_specs, out_specs=out_specs,
                      check_rep=False),
            donate_argnums=donate, keep_unused=True,
        )
        concat_in = [
            np.concatenate([np.asarray(in_maps[c][n]) for c in range(n_cores)],
                           axis=0)
            for n in in_names
        ]
        self._dev_in = jax.block_until_ready(
            [jax.device_put(a, sh) for a in concat_in]
        )
        self._mkzeros = jax.jit(
            lambda: tuple(
                jnp.zeros((n_cores * s[0], *s[1:]), dt) for s, dt in zero_shapes
            ),
            out_shardings=(sh,) * n_outs,
        )

    def run(self):
        jax = self._jax
        zs = self._mkzeros()
        outs = jax.block_until_ready(self._sharded(*self._dev_in, *zs))
        results = []
        hosts = [
            np.asarray(a).reshape(self._n_cores, *self._out_avals[i].shape)
            for i, a in enumerate(outs)
        ]
        for c in range(self._n_cores):
            results.append(
                {name: hosts[i][c] for i, name in enumerate(self._out_names)}
            )
        return _FastResults(results)


_RUNNERS = {}


def _run_hw(nc, in_maps, cfg, trace=False):
    if trace:
        return run_bass_kernel_spmd(
            nc, in_maps, core_ids=list(range(cfg.NC)), trace=True
        )
    key = id(nc)
    if key not in _RUNNERS:
        _RUNNERS[key] = _FastRunner(nc, in_maps, cfg.NC)
    return _RUNNERS[key].run()


def gnn_kernel(x, edge_index, edge_weight, edge_type, n_layers=3, trace=False):
    x = np.asarray(x, dtype=np.float32)
    src = np.asarray(edge_index[0], dtype=np.int64)
    dst = np.asarray(edge_index[1], dtype=np.int64)
    w = np.asarray(edge_weight, dtype=np.float32)

    cfg = Cfg(x.shape[0], x.shape[1], n_layers, 8)
    in_maps, plan = preprocess(x, src, dst, w, cfg)
    nc = build(cfg, plan)
    global _LAST_NC, _LAST_INMAPS, _LAST_CFG
    _LAST_NC, _LAST_INMAPS, _LAST_CFG = nc, in_maps, cfg
    res = _run_hw(nc, in_maps, cfg, trace=trace)

    parts = []
    for r in range(cfg.NC):
        lo = r * cfg.SHARD
        rows = min(cfg.N - lo, cfg.SHARD)
        parts.append(res.results[r]["out"][:rows])
    out = np.concatenate(parts, axis=0)
    return out, res


def kernel(x, edge_index, edge_weight, edge_type):
    out, _ = gnn_kernel(x, edge_index, edge_weight, edge_type)
    return out

